# revision 51
# baseline (speedup 1.0000x reference)
"""Trainium2 Bass kernel for a GPT causal-attention block (v2).

Problem: y = proj(causal_attention(x @ W_attn)), B=4, T=2048, C=1024, 16 heads.
Sharding: 8 cores = 4 batches x 2 head-groups (8 heads each). Each core
computes its batch's attention for its 8 heads plus the partial projection
(W_proj rows of its heads); the host sums the two partials per batch.

v2 design (post-trace analysis of the v1 683us baseline):
  - bf16 operands everywhere (PSUM accumulation stays fp32): halves SBUF/DMA,
    enables FWL fast weight loads. Scores ~N(0,1) -> ~3e-3 rel err, gate 2e-2.
  - S matmuls for the head pair are row-tiled (rows 0-63 / 64-127) into one
    2-bank PSUM tile -> concurrent on the PE; one batched exp [128,1024].
  - Normalization deferred out of the attention inner loop (v1 stalled the
    in-order PE queue on a 3.3us single-partition reciprocal every block).
    Denominator rows -> SBUF, K=1 f32r broadcast matmul, [128,512] reciprocal.
  - Stage-interleaved emission: attention stage i pops quad i+1 qkv work and
    proj(i-1) as PE fillers, keeping the PE dense so HAM stays at 2.4 GHz
    (v1 ran its whole attention phase at K=4/8 = 1.2 GHz).
"""
import sys, os, contextlib
from collections import deque

for _p in ("/opt/trn_rl_repo", "/root/.axon_site/_ro/trn_rl_repo"):
    if os.path.isdir(_p) and _p not in sys.path:
        sys.path.insert(0, _p)

import numpy as np

T, C, NHEAD, HS = 2048, 1024, 16, 64
NCORES = 8
HPC = NHEAD // 2          # heads per core = 8
DPC = HPC * HS            # head dims per core = 512
NCC = C // 128            # contraction chunks = 8
NQT = T // 512            # q tiles = 4
NCH = T // 128            # kt chunks = 16
NPAIR = HPC // 2          # head pairs per core = 4
NQUAD = T // 512          # t quads for transpose/qkv = 4
VLAG = 4                  # S->V chunk pipelining lag (in chunk-pairs)

_CACHE = {}


def _build():
    import concourse.tile as tile
    import concourse.bass as bass
    from concourse import bacc, mybir

    f32 = mybir.dt.float32
    f32r = mybir.dt.float32r
    bf16 = mybir.dt.bfloat16
    FT = mybir.ActivationFunctionType
    from concourse.masks import make_identity

    nc = bacc.Bacc("TRN2", target_bir_lowering=False)
    x_d = nc.declare_dram_parameter("x", [T, C], bf16, isOutput=False)
    wq_d = nc.declare_dram_parameter("wq", [C, DPC], bf16, isOutput=False)
    wk_d = nc.declare_dram_parameter("wk", [C, DPC], bf16, isOutput=False)
    wv_d = nc.declare_dram_parameter("wv", [C, DPC], bf16, isOutput=False)
    wp_d = nc.declare_dram_parameter("wp", [DPC, C], bf16, isOutput=False)
    ones_d = nc.declare_dram_parameter("ones_c", [128, 128], f32, isOutput=False)
    masks_d = nc.declare_dram_parameter("masks_c", [128, 4, 1024], bf16, isOutput=False)
    vones_d = nc.declare_dram_parameter("vones_c", [128, NPAIR, 33], bf16, isOutput=False)
    o_d = nc.declare_dram_parameter("o", [T, C], f32, isOutput=True)

    x_r = x_d[:].rearrange("(n p) c -> n p c", p=128)     # [16, 128, 1024]
    o_r = o_d[:].rearrange("(n p) c -> n p c", p=128)

    with tile.TileContext(nc) as tc:
      with contextlib.ExitStack() as top:
        top.enter_context(nc.allow_low_precision(reason="bf16 pipeline, fp32 accum"))
        const = top.enter_context(tc.tile_pool(name="const", bufs=1))
        persist = top.enter_context(tc.tile_pool(name="persist", bufs=1))
        xpool = top.enter_context(tc.tile_pool(name="xpool", bufs=9))
        xtq_pool = top.enter_context(tc.tile_pool(name="xtq", bufs=1))
        ppool = top.enter_context(tc.tile_pool(name="ppool", bufs=VLAG + 2))
        dpool = top.enter_context(tc.tile_pool(name="dpool", bufs=2))
        dcpool = top.enter_context(tc.tile_pool(name="dcpool", bufs=4))
        opool = top.enter_context(tc.tile_pool(name="opool", bufs=2))
        pss = top.enter_context(tc.tile_pool(name="pss", bufs=2, space="PSUM"))
        psy = top.enter_context(tc.tile_pool(name="psy", bufs=2, space="PSUM"))
        psmm = top.enter_context(tc.tile_pool(name="psmm", bufs=2, space="PSUM"))

        ident = const.tile([128, 128], bf16, tag="ident")
        make_identity(nc, ident)

        # HAM pre-warm: ~32 dummy matmuls (~5us PE-busy) during the initial
        # x/weight DMA wait flip the PE clock gate to 8/8 before real work
        # (transpose-mode does not count as PE activity for the HAM monitor)
        for _w in range(32):
            pwt = psmm.tile([128, 128], f32, tag="mm")
            nc.tensor.matmul(pwt[:], ident[:], ident[:], start=True, stop=True)

        # persistent tensors
        qT = [persist.tile([128, T], bf16, tag=f"qT{u}", name=f"qT{u}") for u in range(NPAIR)]
        kT = [persist.tile([128, T], bf16, tag=f"kT{u}", name=f"kT{u}") for u in range(NPAIR)]
        vp = [persist.tile([128, NPAIR, 161], bf16, tag=f"vp{t}", name=f"vp{t}") for t in range(NCH)]
        yT = [persist.tile([128, T], bf16, tag=f"yT{u}", name=f"yT{u}") for u in range(NPAIR)]

        wq_sb = persist.tile([128, NCC, DPC], bf16, tag="wq")
        wk_sb = persist.tile([128, NCC, DPC], bf16, tag="wk")
        wv_sb = persist.tile([128, NCC, DPC], bf16, tag="wv")
        wp_sb = persist.tile([128, NPAIR, C], bf16, tag="wp")

        # ---------------- qkv quad machinery ----------------
        x_nat = {}     # (quad, j) -> x tile

        def dma_quad_x(q):
            for j in range(4):
                xn = xpool.tile([128, C], bf16, tag="x_nat")
                nc.sync.dma_start(out=xn, in_=x_r[4 * q + j])
                x_nat[(q, j)] = xn

        # startup DMAs in dependency-criticality order: x quad0 feeds the
        # first transposes, then qkv weights, then everything needed later
        dma_quad_x(0)
        nc.sync.dma_start(out=wq_sb, in_=wq_d[:].rearrange("(n p) d -> p n d", p=128))
        nc.sync.dma_start(out=wk_sb, in_=wk_d[:].rearrange("(n p) d -> p n d", p=128))
        nc.sync.dma_start(out=wv_sb, in_=wv_d[:].rearrange("(n p) d -> p n d", p=128))
        ones_row = const.tile([128, 128], f32r, tag="ones_row")
        nc.sync.dma_start(out=ones_row, in_=ones_d[:].bitcast(f32r))
        masks = const.tile([128, 4, 1024], bf16, tag="masks")
        nc.sync.dma_start(out=masks, in_=masks_d[:])
        for t in range(NCH):
            nc.sync.dma_start(out=vp[t][:, :, 64:97], in_=vones_d[:])
        nc.sync.dma_start(out=wp_sb, in_=wp_d[:].rearrange("(n p) c -> p n c", p=128))

        xTq = {}       # cc -> tile (rewritten each quad)

        def quad_closures(q):
            """List of (label, closure) emitting quad q's transposes + qkv."""
            ops = []
            qs = slice(512 * q, 512 * (q + 1))

            def mk_tr(cc):
                def f():
                    pst = psmm.tile([128, 512], bf16, tag="mm")
                    for j in range(4):
                        nc.tensor.transpose(pst[:, j * 128:(j + 1) * 128],
                                            x_nat[(q, j)][:, cc * 128:(cc + 1) * 128],
                                            ident)
                    xt = xtq_pool.tile([128, 512], bf16, tag=f"xTq{cc}", name=f"xTq{cc}")
                    nc.vector.tensor_copy(xt[:], pst[:])
                    xTq[cc] = xt
                return f

            def mk_qk(dt, which):
                w_sb, dst = (wq_sb, qT) if which == "q" else (wk_sb, kT)

                def f():
                    ps = psmm.tile([128, 512], f32, tag="mm")
                    for cc in range(NCC):
                        nc.tensor.matmul(ps[:], w_sb[:, cc, dt * 128:(dt + 1) * 128],
                                         xTq[cc][:], start=(cc == 0), stop=(cc == NCC - 1))
                    nc.vector.tensor_copy(dst[dt][:, qs], ps[:])
                return f

            def mk_v(j):
                tt = 4 * q + j

                def f():
                    ps = psmm.tile([128, 512], f32, tag="mm")
                    for cc in range(NCC):
                        nc.tensor.matmul(ps[:], xTq[cc][:, j * 128:(j + 1) * 128],
                                         wv_sb[:, cc, :], start=(cc == 0), stop=(cc == NCC - 1))
                    pv = ps.rearrange("p (u two d) -> p u two d", u=NPAIR, two=2)
                    nc.vector.tensor_copy(vp[tt][:, :, 0:64], pv[:, :, 0, :])
                    nc.vector.tensor_copy(vp[tt][:, :, 97:161], pv[:, :, 1, :])
                return f

            for cc in range(NCC):
                ops.append((("tr", q, cc), mk_tr(cc)))
            for dt in range(4):
                ops.append((("qk", q, dt, "q"), mk_qk(dt, "q")))
                ops.append((("qk", q, dt, "k"), mk_qk(dt, "k")))
            for j in range(4):
                ops.append((("v", q, j), mk_v(j)))
            return ops

        # ---------------- attention block ----------------
        def mk_normalize(u, i, dc):
            # dc rows 64*(u%2) / +32 hold 1/d for pair u's even/odd head
            qs = slice(512 * i, 512 * (i + 1))
            pe_, po_ = 64 * (u % 2), 64 * (u % 2) + 32

            def f():
                rb1 = psmm.tile([128, 512], f32, tag="mm")
                nc.tensor.matmul(rb1[:, :], ones_row[pe_:pe_ + 1, :],
                                 dc[pe_:pe_ + 1, :], start=True, stop=True,
                                 tile_position=(pe_, 0))
                rb2 = psmm.tile([128, 512], f32, tag="mm")
                nc.tensor.matmul(rb2[:, :], ones_row[po_:po_ + 1, :],
                                 dc[po_:po_ + 1, :], start=True, stop=True,
                                 tile_position=(po_, 0))
                nc.vector.tensor_mul(yT[u][0:64, qs], yT[u][0:64, qs], rb1[0:64, :])
                nc.vector.tensor_mul(yT[u][64:128, qs], yT[u][64:128, qs], rb2[64:128, :])
            return f

        def emit_attn(u, i, dc, pop):
            L = 4 * (i + 1)
            qs = slice(512 * i, 512 * (i + 1))
            ps_e = psy.tile([128, 512], f32, tag="ps_y")
            ps_o = psy.tile([128, 512], f32, tag="ps_y")
            Pt = {}

            def pv(c):
                nc.tensor.matmul(ps_e[0:65, :], vp[c][:, u, 0:65], Pt[c][:, 0:512],
                                 start=(c == 0), stop=(c == L - 1))
                nc.tensor.matmul(ps_o[:, :], vp[c][:, u, 33:161], Pt[c][:, 512:1024],
                                 start=(c == 0), stop=(c == L - 1))

            for c in range(L):
                sp = pss.tile([128, 1024], f32, tag="s_pair")
                nc.tensor.matmul(sp[:, 0:512], kT[u][0:64, c * 128:(c + 1) * 128],
                                 qT[u][0:64, qs], start=True, stop=True)
                nc.tensor.matmul(sp[:, 512:1024], kT[u][64:128, c * 128:(c + 1) * 128],
                                 qT[u][64:128, qs], start=True, stop=True)
                P = ppool.tile([128, 1024], bf16, tag="P")
                nc.scalar.activation(out=P[:], in_=sp[:], func=FT.Exp,
                                     scale=float(HS) ** -0.5)
                if c >= 4 * i:
                    nc.vector.tensor_mul(P[:], P[:], masks[:, c - 4 * i, :])
                Pt[c] = P
                if c >= VLAG:
                    pv(c - VLAG)
                pop()
            for c in range(max(0, L - VLAG), L):
                pv(c)
            # epilogue: denominator rows first (they head the normalize
            # chain), then the unnormalized yT copies
            d_sb = dpool.tile([128, 512], f32r, tag="d_sb")
            nc.vector.tensor_copy(d_sb[64:65, :], ps_e[64:65, :].bitcast(f32r))
            nc.vector.tensor_copy(d_sb[32:33, :], ps_o[32:33, :].bitcast(f32r))
            # stage the d rows into 32-aligned collector rows (partition-
            # shifting SBUF->SBUF DMA); one batched reciprocal per u-pair
            pe_ = 64 * (u % 2)
            nc.sync.dma_start(out=dc[pe_:pe_ + 1, :], in_=d_sb[64:65, :])
            nc.sync.dma_start(out=dc[pe_ + 32:pe_ + 33, :], in_=d_sb[32:33, :])
            nc.vector.tensor_copy(yT[u][0:64, qs], ps_e[0:64, :])
            nc.vector.tensor_copy(yT[u][64:128, qs], ps_o[64:128, :])

        # ---------------- projection ----------------
        def proj_closures(i, on_scalar=False):
            ops = []

            def mk_proj(tt):
                def f():
                    out_sb = opool.tile([128, C], f32, tag="out_sb")
                    for ct in range(2):
                        po = psmm.tile([128, 512], f32, tag="mm")
                        for u in range(NPAIR):
                            nc.tensor.matmul(po[:], yT[u][:, tt * 128:(tt + 1) * 128],
                                             wp_sb[:, u, ct * 512:(ct + 1) * 512],
                                             start=(u == 0), stop=(u == NPAIR - 1))
                        nc.vector.tensor_copy(out_sb[:, ct * 512:(ct + 1) * 512], po[:])
                    nc.sync.dma_start(out=o_r[tt], in_=out_sb)
                return f

            for tt in range(4 * i, 4 * i + 4):
                ops.append((("proj", i, tt), mk_proj(tt)))
            return ops

        # ---------------- emission schedule ----------------
        q0 = quad_closures(0)
        # prologue: quad0 transposes + pair-0 q/k + all v closures (attn(0,0) deps)
        prologue_keys = {("tr", 0, cc) for cc in range(NCC)}
        prologue_keys |= {("qk", 0, 0, "q"), ("qk", 0, 0, "k")}
        prologue_keys |= {("v", 0, j) for j in range(4)}
        rest0 = []
        for key, cl in q0:
            if key in prologue_keys:
                cl()
                if key[0] == "tr":
                    # transposes don't register as PE activity for the HAM
                    # clock-gate monitor; keep it fed with real matmuls so
                    # the 2.4 GHz state survives until qkv work arrives
                    for _w2 in range(3):
                        pwt2 = psmm.tile([128, 128], f32, tag="mm")
                        nc.tensor.matmul(pwt2[:], ident[:], ident[:],
                                         start=True, stop=True)
            else:
                rest0.append((key, cl))

        fillers = deque(rest0)

        def make_pop(total_pairs, spill=0):
            # spill > 0 spreads fillers past the stage end so some PE work
            # flows into the next (more ACT-bound) stage / the tail
            state = {"pairs_left": total_pairs + spill}

            def pop():
                # spread remaining fillers over remaining chunk-pairs; floor
                # division so filler work survives to the stage end (the PE
                # would otherwise starve there and HAM re-throttles)
                n = state["pairs_left"]
                if n > 0:
                    k = len(fillers) // n
                    for _ in range(min(k, len(fillers))):
                        key, cl = fillers.popleft()
                        cl()
                    state["pairs_left"] = n - 1
            return pop

        dma_quad_x(1)   # prefetch: x quads issue 2 stages ahead so they sit
        for i in range(NQT):    # in the sync queue before the d-row DMA waits
            if i < NQT - 2:
                dma_quad_x(i + 2)
            if i < NQT - 1:
                fillers.extend(quad_closures(i + 1))
            if i == NQT - 1:
                # all earlier projections run as last-stage fillers: that
                # stage is ACT-bound and flips to PE-bound-cold if starved
                for j in range(NQT - 1):
                    fillers.extend(proj_closures(j))
            pop = make_pop(NPAIR * 4 * (i + 1), spill=12 if i < 3 else 6)
            dcs = [dcpool.tile([128, 512], f32r, tag="dc", name=f"dc{i}_{h}")
                   for h in range(2)]
            for dc in dcs:
                nc.gpsimd.memset(dc[:].bitcast(f32), 1.0)
            for u in range(NPAIR):
                # force-emit producers this attention block depends on
                need = {("qk", qq, u, w) for qq in range(i + 1) for w in ("q", "k")}
                need |= {("v", qq, j) for qq in range(i + 1) for j in range(4)}
                need |= {("tr", qq, cc) for qq in range(i + 1) for cc in range(NCC)}
                while any(key in need for key, _ in fillers):
                    key, cl = fillers.popleft()
                    cl()
                emit_attn(u, i, dcs[u // 2], pop)
                if u % 2 == 1:
                    # batched reciprocal for this u-pair; normalize becomes
                    # filler work (keeps the PE dense at stage boundaries)
                    nc.vector.reciprocal(dcs[u // 2][:], dcs[u // 2][:])
                    fillers.append((("norm", u - 1, i), mk_normalize(u - 1, i, dcs[u // 2])))
                    fillers.append((("norm", u, i), mk_normalize(u, i, dcs[u // 2])))
        # tail: drain leftovers, then final projection
        while fillers:
            key, cl = fillers.popleft()
            cl()
        for key, cl in proj_closures(NQT - 1, on_scalar=True):
            cl()

    nc.compile()
    return nc


def _get_nc():
    if "nc" not in _CACHE:
        _CACHE["nc"] = _build()
    return _CACHE["nc"]


def _in_maps(x, W_attn, W_proj):
    import ml_dtypes
    bf16 = ml_dtypes.bfloat16
    ones_c = np.ones((128, 128), np.float32)
    a_idx = np.arange(128)[:, None]
    b_idx = np.arange(512)[None, :]
    m = np.stack([(b_idx - a_idx - 128 * j >= 0) for j in range(4)], 0)  # [4,128,512]
    m = m.transpose(1, 0, 2).astype(np.float32)                          # [128,4,512]
    masks_c = np.ascontiguousarray(
        np.concatenate([m, m], axis=2).astype(bf16))                     # [128,4,1024]
    vones_c = np.ones((128, NPAIR, 33), bf16)
    maps = []
    for core in range(NCORES):
        b, g = core // 2, core % 2
        cs = slice(DPC * g, DPC * (g + 1))
        maps.append({
            "x": np.ascontiguousarray(x[b].astype(bf16)),
            "wq": np.ascontiguousarray(W_attn[:, cs].astype(bf16)),
            "wk": np.ascontiguousarray(W_attn[:, C:][:, cs].astype(bf16)),
            "wv": np.ascontiguousarray(W_attn[:, 2 * C:][:, cs].astype(bf16)),
            "wp": np.ascontiguousarray(W_proj[cs, :].astype(bf16)),
            "ones_c": ones_c,
            "masks_c": masks_c,
            "vones_c": vones_c,
        })
    return maps


def _install_ntff_shim():
    """Provide antenv.axon_hooks (absent in this image) so trace=True works."""
    import sys as _sys, types, ctypes, contextlib as _cl
    if "antenv.axon_hooks" in _sys.modules:
        return
    so_path = "/opt/axon/libaxon_pjrt.so"
    try:
        lib = ctypes.CDLL(so_path)
        lib.axon_start_nrt_profile.argtypes = [ctypes.POINTER(ctypes.c_int64), ctypes.c_size_t]
        lib.axon_start_nrt_profile.restype = ctypes.c_int64
        lib.axon_stop_nrt_profile.argtypes = [ctypes.c_char_p]
        lib.axon_stop_nrt_profile.restype = ctypes.c_int64
    except (OSError, AttributeError):
        return

    @_cl.contextmanager
    def _hook(output_dir, device_ids):
        import jax
        jax.devices()
        if device_ids:
            ids = (ctypes.c_int64 * len(device_ids))(*device_ids)
            rc = lib.axon_start_nrt_profile(ids, len(device_ids))
        else:
            rc = lib.axon_start_nrt_profile(None, 0)
        if rc != 0:
            raise RuntimeError(f"axon_start_nrt_profile rc={rc}")
        try:
            yield
        finally:
            n = lib.axon_stop_nrt_profile(str(output_dir).encode())
            if n < 0:
                raise RuntimeError(f"axon_stop_nrt_profile rc={n}")

    mod = types.ModuleType("antenv.axon_hooks")
    mod.get_axon_ntff_profile_hook = lambda: _hook
    mod.set_axon_ntff_profile_hook = lambda h: None
    _sys.modules["antenv.axon_hooks"] = mod


def kernel(x, W_attn, W_proj, _trace=False):
    from concourse.bass_utils import run_bass_kernel_spmd
    if _trace:
        _install_ntff_shim()
    x = np.asarray(x, dtype=np.float32)
    W_attn = np.asarray(W_attn, dtype=np.float32)
    W_proj = np.asarray(W_proj, dtype=np.float32)
    nc = _get_nc()
    res = run_bass_kernel_spmd(nc, _in_maps(x, W_attn, W_proj),
                               core_ids=list(range(NCORES)), trace=_trace)
    out = np.empty((4, T, C), np.float32)
    for b in range(4):
        out[b] = res.results[2 * b]["o"] + res.results[2 * b + 1]["o"]
    if _trace:
        return out, res
    return out


# revision 53
# speedup vs baseline: 1.0203x; 1.0203x over previous
"""Trainium2 Bass kernel for a GPT causal-attention block (v2).

Problem: y = proj(causal_attention(x @ W_attn)), B=4, T=2048, C=1024, 16 heads.
Sharding: 8 cores = 4 batches x 2 head-groups (8 heads each). Each core
computes its batch's attention for its 8 heads plus the partial projection
(W_proj rows of its heads); the host sums the two partials per batch.

v2 design (post-trace analysis of the v1 683us baseline):
  - bf16 operands everywhere (PSUM accumulation stays fp32): halves SBUF/DMA,
    enables FWL fast weight loads. Scores ~N(0,1) -> ~3e-3 rel err, gate 2e-2.
  - S matmuls for the head pair are row-tiled (rows 0-63 / 64-127) into one
    2-bank PSUM tile -> concurrent on the PE; one batched exp [128,1024].
  - Normalization deferred out of the attention inner loop (v1 stalled the
    in-order PE queue on a 3.3us single-partition reciprocal every block).
    Denominator rows -> SBUF, K=1 f32r broadcast matmul, [128,512] reciprocal.
  - Stage-interleaved emission: attention stage i pops quad i+1 qkv work and
    proj(i-1) as PE fillers, keeping the PE dense so HAM stays at 2.4 GHz
    (v1 ran its whole attention phase at K=4/8 = 1.2 GHz).
"""
import sys, os, contextlib
from collections import deque

for _p in ("/opt/trn_rl_repo", "/root/.axon_site/_ro/trn_rl_repo"):
    if os.path.isdir(_p) and _p not in sys.path:
        sys.path.insert(0, _p)

import numpy as np

T, C, NHEAD, HS = 2048, 1024, 16, 64
NCORES = 8
HPC = NHEAD // 2          # heads per core = 8
DPC = HPC * HS            # head dims per core = 512
NCC = C // 128            # contraction chunks = 8
NQT = T // 512            # q tiles = 4
NCH = T // 128            # kt chunks = 16
NPAIR = HPC // 2          # head pairs per core = 4
NQUAD = T // 512          # t quads for transpose/qkv = 4
VLAG = 4                  # S->V chunk pipelining lag (in chunk-pairs)

_CACHE = {}


def _build():
    import concourse.tile as tile
    import concourse.bass as bass
    from concourse import bacc, mybir

    f32 = mybir.dt.float32
    f32r = mybir.dt.float32r
    bf16 = mybir.dt.bfloat16
    FT = mybir.ActivationFunctionType
    from concourse.masks import make_identity

    nc = bacc.Bacc("TRN2", target_bir_lowering=False)
    x_d = nc.declare_dram_parameter("x", [T, C], bf16, isOutput=False)
    wq_d = nc.declare_dram_parameter("wq", [C, DPC], bf16, isOutput=False)
    wk_d = nc.declare_dram_parameter("wk", [C, DPC], bf16, isOutput=False)
    wv_d = nc.declare_dram_parameter("wv", [C, DPC], bf16, isOutput=False)
    wp_d = nc.declare_dram_parameter("wp", [DPC, C], bf16, isOutput=False)
    ones_d = nc.declare_dram_parameter("ones_c", [128, 128], f32, isOutput=False)
    masks_d = nc.declare_dram_parameter("masks_c", [128, 4, 1024], bf16, isOutput=False)
    vones_d = nc.declare_dram_parameter("vones_c", [128, NPAIR, 33], bf16, isOutput=False)
    o_d = nc.declare_dram_parameter("o", [T, C], f32, isOutput=True)

    x_r = x_d[:].rearrange("(n p) c -> n p c", p=128)     # [16, 128, 1024]
    o_r = o_d[:].rearrange("(n p) c -> n p c", p=128)

    with tile.TileContext(nc) as tc:
      with contextlib.ExitStack() as top:
        top.enter_context(nc.allow_low_precision(reason="bf16 pipeline, fp32 accum"))
        const = top.enter_context(tc.tile_pool(name="const", bufs=1))
        persist = top.enter_context(tc.tile_pool(name="persist", bufs=1))
        xpool = top.enter_context(tc.tile_pool(name="xpool", bufs=9))
        xtq_pool = top.enter_context(tc.tile_pool(name="xtq", bufs=2))
        ppool = top.enter_context(tc.tile_pool(name="ppool", bufs=VLAG + 4))
        dpool = top.enter_context(tc.tile_pool(name="dpool", bufs=2))
        dcpool = top.enter_context(tc.tile_pool(name="dcpool", bufs=4))
        opool = top.enter_context(tc.tile_pool(name="opool", bufs=2))
        pss = top.enter_context(tc.tile_pool(name="pss", bufs=2, space="PSUM"))
        psy = top.enter_context(tc.tile_pool(name="psy", bufs=2, space="PSUM"))
        psmm = top.enter_context(tc.tile_pool(name="psmm", bufs=2, space="PSUM"))

        ident = const.tile([128, 128], bf16, tag="ident")
        make_identity(nc, ident)

        # HAM pre-warm: ~32 dummy matmuls (~5us PE-busy) during the initial
        # x/weight DMA wait flip the PE clock gate to 8/8 before real work
        # (transpose-mode does not count as PE activity for the HAM monitor)
        for _w in range(32):
            pwt = psmm.tile([128, 128], f32, tag="mm")
            nc.tensor.matmul(pwt[:], ident[:], ident[:], start=True, stop=True)

        # persistent tensors
        qT = [persist.tile([128, T], bf16, tag=f"qT{u}", name=f"qT{u}") for u in range(NPAIR)]
        kT = [persist.tile([128, T], bf16, tag=f"kT{u}", name=f"kT{u}") for u in range(NPAIR)]
        vp = [persist.tile([128, NPAIR, 161], bf16, tag=f"vp{t}", name=f"vp{t}") for t in range(NCH)]
        yT = [persist.tile([128, T], bf16, tag=f"yT{u}", name=f"yT{u}") for u in range(NPAIR)]

        wq_sb = persist.tile([128, NCC, DPC], bf16, tag="wq")
        wk_sb = persist.tile([128, NCC, DPC], bf16, tag="wk")
        wv_sb = persist.tile([128, NCC, DPC], bf16, tag="wv")
        wp_sb = persist.tile([128, NPAIR, C], bf16, tag="wp")

        # ---------------- qkv quad machinery ----------------
        x_nat = {}     # (quad, j) -> x tile

        def dma_quad_x(q):
            for j in range(4):
                xn = xpool.tile([128, C], bf16, tag="x_nat")
                nc.sync.dma_start(out=xn, in_=x_r[4 * q + j])
                x_nat[(q, j)] = xn

        # startup DMAs in dependency-criticality order: x quad0 feeds the
        # first transposes, then qkv weights, then everything needed later
        dma_quad_x(0)
        nc.sync.dma_start(out=wq_sb, in_=wq_d[:].rearrange("(n p) d -> p n d", p=128))
        nc.sync.dma_start(out=wk_sb, in_=wk_d[:].rearrange("(n p) d -> p n d", p=128))
        nc.sync.dma_start(out=wv_sb, in_=wv_d[:].rearrange("(n p) d -> p n d", p=128))
        ones_row = const.tile([128, 128], f32r, tag="ones_row")
        nc.sync.dma_start(out=ones_row, in_=ones_d[:].bitcast(f32r))
        masks = const.tile([128, 4, 1024], bf16, tag="masks")
        nc.sync.dma_start(out=masks, in_=masks_d[:])
        for t in range(NCH):
            nc.sync.dma_start(out=vp[t][:, :, 64:97], in_=vones_d[:])
        nc.sync.dma_start(out=wp_sb, in_=wp_d[:].rearrange("(n p) c -> p n c", p=128))

        xTq = {}       # cc -> tile (rewritten each quad)

        def quad_closures(q):
            """List of (label, closure) emitting quad q's transposes + qkv."""
            ops = []
            qs = slice(512 * q, 512 * (q + 1))

            def mk_tr(cc):
                def f():
                    pst = psmm.tile([128, 512], bf16, tag="mm")
                    for j in range(4):
                        nc.tensor.transpose(pst[:, j * 128:(j + 1) * 128],
                                            x_nat[(q, j)][:, cc * 128:(cc + 1) * 128],
                                            ident)
                    xt = xtq_pool.tile([128, 512], bf16, tag=f"xTq{cc}", name=f"xTq{cc}")
                    nc.vector.tensor_copy(xt[:], pst[:])
                    xTq[cc] = xt
                return f

            def mk_qk(dt, which):
                w_sb, dst = (wq_sb, qT) if which == "q" else (wk_sb, kT)

                def f():
                    ps = psmm.tile([128, 512], f32, tag="mm")
                    for cc in range(NCC):
                        nc.tensor.matmul(ps[:], w_sb[:, cc, dt * 128:(dt + 1) * 128],
                                         xTq[cc][:], start=(cc == 0), stop=(cc == NCC - 1))
                    nc.vector.tensor_copy(dst[dt][:, qs], ps[:])
                return f

            def mk_v(j):
                tt = 4 * q + j

                def f():
                    ps = psmm.tile([128, 512], f32, tag="mm")
                    for cc in range(NCC):
                        nc.tensor.matmul(ps[:], xTq[cc][:, j * 128:(j + 1) * 128],
                                         wv_sb[:, cc, :], start=(cc == 0), stop=(cc == NCC - 1))
                    pv = ps.rearrange("p (u two d) -> p u two d", u=NPAIR, two=2)
                    nc.vector.tensor_copy(vp[tt][:, :, 0:64], pv[:, :, 0, :])
                    nc.vector.tensor_copy(vp[tt][:, :, 97:161], pv[:, :, 1, :])
                return f

            for cc in range(NCC):
                ops.append((("tr", q, cc), mk_tr(cc)))
            for dt in range(4):
                ops.append((("qk", q, dt, "q"), mk_qk(dt, "q")))
                ops.append((("qk", q, dt, "k"), mk_qk(dt, "k")))
            for j in range(4):
                ops.append((("v", q, j), mk_v(j)))
            return ops

        # ---------------- attention block ----------------
        def mk_normalize(u, i, dc):
            # dc rows 64*(u%2) / +32 hold 1/d for pair u's even/odd head
            qs = slice(512 * i, 512 * (i + 1))
            pe_, po_ = 64 * (u % 2), 64 * (u % 2) + 32

            def f():
                rb1 = psmm.tile([128, 512], f32, tag="mm")
                nc.tensor.matmul(rb1[:, :], ones_row[pe_:pe_ + 1, :],
                                 dc[pe_:pe_ + 1, :], start=True, stop=True,
                                 tile_position=(pe_, 0))
                rb2 = psmm.tile([128, 512], f32, tag="mm")
                nc.tensor.matmul(rb2[:, :], ones_row[po_:po_ + 1, :],
                                 dc[po_:po_ + 1, :], start=True, stop=True,
                                 tile_position=(po_, 0))
                nc.vector.tensor_mul(yT[u][0:64, qs], yT[u][0:64, qs], rb1[0:64, :])
                nc.vector.tensor_mul(yT[u][64:128, qs], yT[u][64:128, qs], rb2[64:128, :])
            return f

        def emit_attn(u, i, dc, pop):
            L = 4 * (i + 1)
            qs = slice(512 * i, 512 * (i + 1))
            ps_e = psy.tile([128, 512], f32, tag="ps_y")
            ps_o = psy.tile([128, 512], f32, tag="ps_y")
            Pt = {}

            def pv(c):
                nc.tensor.matmul(ps_e[0:65, :], vp[c][:, u, 0:65], Pt[c][:, 0:512],
                                 start=(c == 0), stop=(c == L - 1))
                nc.tensor.matmul(ps_o[:, :], vp[c][:, u, 33:161], Pt[c][:, 512:1024],
                                 start=(c == 0), stop=(c == L - 1))

            for c in range(L):
                sp = pss.tile([128, 1024], f32, tag="s_pair")
                nc.tensor.matmul(sp[:, 0:512], kT[u][0:64, c * 128:(c + 1) * 128],
                                 qT[u][0:64, qs], start=True, stop=True)
                nc.tensor.matmul(sp[:, 512:1024], kT[u][64:128, c * 128:(c + 1) * 128],
                                 qT[u][64:128, qs], start=True, stop=True)
                P = ppool.tile([128, 1024], bf16, tag="P")
                nc.scalar.activation(out=P[:], in_=sp[:], func=FT.Exp,
                                     scale=float(HS) ** -0.5)
                if c >= 4 * i:
                    nc.vector.tensor_mul(P[:], P[:], masks[:, c - 4 * i, :])
                Pt[c] = P
                if c >= VLAG:
                    pv(c - VLAG)
                pop()
            for c in range(max(0, L - VLAG), L):
                pv(c)
            # epilogue: denominator rows first (they head the normalize
            # chain), then the unnormalized yT copies
            d_sb = dpool.tile([128, 512], f32r, tag="d_sb")
            nc.vector.tensor_copy(d_sb[64:65, :], ps_e[64:65, :].bitcast(f32r))
            nc.vector.tensor_copy(d_sb[32:33, :], ps_o[32:33, :].bitcast(f32r))
            # stage the d rows into 32-aligned collector rows (partition-
            # shifting SBUF->SBUF DMA); one batched reciprocal per u-pair
            pe_ = 64 * (u % 2)
            nc.sync.dma_start(out=dc[pe_:pe_ + 1, :], in_=d_sb[64:65, :])
            nc.sync.dma_start(out=dc[pe_ + 32:pe_ + 33, :], in_=d_sb[32:33, :])
            nc.vector.tensor_copy(yT[u][0:64, qs], ps_e[0:64, :])
            nc.vector.tensor_copy(yT[u][64:128, qs], ps_o[64:128, :])

        # ---------------- projection ----------------
        def proj_closures(i, on_scalar=False):
            ops = []

            def mk_proj(tt):
                def f():
                    out_sb = opool.tile([128, C], f32, tag="out_sb")
                    for ct in range(2):
                        po = psmm.tile([128, 512], f32, tag="mm")
                        for u in range(NPAIR):
                            nc.tensor.matmul(po[:], yT[u][:, tt * 128:(tt + 1) * 128],
                                             wp_sb[:, u, ct * 512:(ct + 1) * 512],
                                             start=(u == 0), stop=(u == NPAIR - 1))
                        nc.vector.tensor_copy(out_sb[:, ct * 512:(ct + 1) * 512], po[:])
                    nc.sync.dma_start(out=o_r[tt], in_=out_sb)
                return f

            for tt in range(4 * i, 4 * i + 4):
                ops.append((("proj", i, tt), mk_proj(tt)))
            return ops

        # ---------------- emission schedule ----------------
        q0 = quad_closures(0)
        # prologue: quad0 transposes + pair-0 q/k + all v closures (attn(0,0) deps)
        prologue_keys = {("tr", 0, cc) for cc in range(NCC)}
        prologue_keys |= {("qk", 0, 0, "q"), ("qk", 0, 0, "k")}
        prologue_keys |= {("v", 0, j) for j in range(4)}
        rest0 = []
        for key, cl in q0:
            if key in prologue_keys:
                cl()
            else:
                rest0.append((key, cl))

        fillers = deque(rest0)

        def make_pop(total_pairs, spill=0):
            # spill > 0 spreads fillers past the stage end so some PE work
            # flows into the next (more ACT-bound) stage / the tail
            state = {"pairs_left": total_pairs + spill}

            def pop():
                # spread remaining fillers over remaining chunk-pairs; floor
                # division so filler work survives to the stage end (the PE
                # would otherwise starve there and HAM re-throttles)
                n = state["pairs_left"]
                if n > 0:
                    k = len(fillers) // n
                    for _ in range(min(k, len(fillers))):
                        key, cl = fillers.popleft()
                        cl()
                    state["pairs_left"] = n - 1
            return pop

        dma_quad_x(1)   # prefetch: x quads issue 2 stages ahead so they sit
        for i in range(NQT):    # in the sync queue before the d-row DMA waits
            if i < NQT - 2:
                dma_quad_x(i + 2)
            if i < NQT - 1:
                fillers.extend(quad_closures(i + 1))
            if i == NQT - 1:
                # all earlier projections run as last-stage fillers: that
                # stage is ACT-bound and flips to PE-bound-cold if starved
                for j in range(NQT - 1):
                    fillers.extend(proj_closures(j))
            pop = make_pop(NPAIR * 4 * (i + 1), spill=12 if i < 3 else 6)
            dcs = [dcpool.tile([128, 512], f32r, tag="dc", name=f"dc{i}_{h}")
                   for h in range(2)]
            for dc in dcs:
                nc.gpsimd.memset(dc[:].bitcast(f32), 1.0)
            for u in range(NPAIR):
                # force-emit producers this attention block depends on
                need = {("qk", qq, u, w) for qq in range(i + 1) for w in ("q", "k")}
                need |= {("v", qq, j) for qq in range(i + 1) for j in range(4)}
                need |= {("tr", qq, cc) for qq in range(i + 1) for cc in range(NCC)}
                while any(key in need for key, _ in fillers):
                    key, cl = fillers.popleft()
                    cl()
                emit_attn(u, i, dcs[u // 2], pop)
                if u % 2 == 1:
                    # batched reciprocal for this u-pair; normalize becomes
                    # filler work (keeps the PE dense at stage boundaries)
                    nc.vector.reciprocal(dcs[u // 2][:], dcs[u // 2][:])
                    fillers.append((("norm", u - 1, i), mk_normalize(u - 1, i, dcs[u // 2])))
                    fillers.append((("norm", u, i), mk_normalize(u, i, dcs[u // 2])))
        # tail: drain leftovers, then final projection
        while fillers:
            key, cl = fillers.popleft()
            cl()
        for key, cl in proj_closures(NQT - 1, on_scalar=True):
            cl()

    nc.compile()
    return nc


def _get_nc():
    if "nc" not in _CACHE:
        _CACHE["nc"] = _build()
    return _CACHE["nc"]


def _in_maps(x, W_attn, W_proj):
    import ml_dtypes
    bf16 = ml_dtypes.bfloat16
    ones_c = np.ones((128, 128), np.float32)
    a_idx = np.arange(128)[:, None]
    b_idx = np.arange(512)[None, :]
    m = np.stack([(b_idx - a_idx - 128 * j >= 0) for j in range(4)], 0)  # [4,128,512]
    m = m.transpose(1, 0, 2).astype(np.float32)                          # [128,4,512]
    masks_c = np.ascontiguousarray(
        np.concatenate([m, m], axis=2).astype(bf16))                     # [128,4,1024]
    vones_c = np.ones((128, NPAIR, 33), bf16)
    maps = []
    for core in range(NCORES):
        b, g = core // 2, core % 2
        cs = slice(DPC * g, DPC * (g + 1))
        maps.append({
            "x": np.ascontiguousarray(x[b].astype(bf16)),
            "wq": np.ascontiguousarray(W_attn[:, cs].astype(bf16)),
            "wk": np.ascontiguousarray(W_attn[:, C:][:, cs].astype(bf16)),
            "wv": np.ascontiguousarray(W_attn[:, 2 * C:][:, cs].astype(bf16)),
            "wp": np.ascontiguousarray(W_proj[cs, :].astype(bf16)),
            "ones_c": ones_c,
            "masks_c": masks_c,
            "vones_c": vones_c,
        })
    return maps


def _install_ntff_shim():
    """Provide antenv.axon_hooks (absent in this image) so trace=True works."""
    import sys as _sys, types, ctypes, contextlib as _cl
    if "antenv.axon_hooks" in _sys.modules:
        return
    so_path = "/opt/axon/libaxon_pjrt.so"
    try:
        lib = ctypes.CDLL(so_path)
        lib.axon_start_nrt_profile.argtypes = [ctypes.POINTER(ctypes.c_int64), ctypes.c_size_t]
        lib.axon_start_nrt_profile.restype = ctypes.c_int64
        lib.axon_stop_nrt_profile.argtypes = [ctypes.c_char_p]
        lib.axon_stop_nrt_profile.restype = ctypes.c_int64
    except (OSError, AttributeError):
        return

    @_cl.contextmanager
    def _hook(output_dir, device_ids):
        import jax
        jax.devices()
        if device_ids:
            ids = (ctypes.c_int64 * len(device_ids))(*device_ids)
            rc = lib.axon_start_nrt_profile(ids, len(device_ids))
        else:
            rc = lib.axon_start_nrt_profile(None, 0)
        if rc != 0:
            raise RuntimeError(f"axon_start_nrt_profile rc={rc}")
        try:
            yield
        finally:
            n = lib.axon_stop_nrt_profile(str(output_dir).encode())
            if n < 0:
                raise RuntimeError(f"axon_stop_nrt_profile rc={n}")

    mod = types.ModuleType("antenv.axon_hooks")
    mod.get_axon_ntff_profile_hook = lambda: _hook
    mod.set_axon_ntff_profile_hook = lambda h: None
    _sys.modules["antenv.axon_hooks"] = mod


def kernel(x, W_attn, W_proj, _trace=False):
    from concourse.bass_utils import run_bass_kernel_spmd
    if _trace:
        _install_ntff_shim()
    x = np.asarray(x, dtype=np.float32)
    W_attn = np.asarray(W_attn, dtype=np.float32)
    W_proj = np.asarray(W_proj, dtype=np.float32)
    nc = _get_nc()
    res = run_bass_kernel_spmd(nc, _in_maps(x, W_attn, W_proj),
                               core_ids=list(range(NCORES)), trace=_trace)
    out = np.empty((4, T, C), np.float32)
    for b in range(4):
        out[b] = res.results[2 * b]["o"] + res.results[2 * b + 1]["o"]
    if _trace:
        return out, res
    return out
